# revision 1
# baseline (speedup 1.0000x reference)
"""LCNNConv2d (dictionary 1x1 conv + sparse lookup combine) on 8 TRN2 NeuronCores.

Math: out[b,o,h,w] = sum_d w2[o,d] * sum_c dict[d,c] * x[b,c,h,w]
                   = sum_c (w2 @ dict)[o,c] * x[b,c,h,w]
with w2 the [O,D] scatter of lookup_coefficients at lookup_indices.

The [O=256, C=64] effective weight is tiny, so it is folded on the host; the
device kernel is a memory-bound streaming matmul, data-parallel over batch:
core i handles x[2i:2i+2] (read 8.4MB, write 33.6MB per core).

Per-core layout trick: the shard [2, 64, 16384] is viewed as [128, 16384]
(partition p = 64*b + c), so every DMA moves full-128-partition tiles. Two
zero-padded stationary weights (rows 0:64 <- W_eff.T for batch 0; rows 64:128
for batch 1) select the right batch during the 128-deep contraction.

DMA plumbing (the actual bottleneck): input loads go through SWDGE (gpsimd)
while output stores alternate between the two HWDGE rings (scalar / sync), so
three DMA issue FIFOs run concurrently.
"""

import numpy as np

B, C_IN, H, W = 16, 64, 128, 128
C_OUT, D_SIZE, SPARSITY = 256, 512, 4
N_CORES = 8
BPC = B // N_CORES           # batches per core = 2
HW = H * W                   # 16384
G = 1024                     # hw columns per tile (512KB DMAs)
PSW = 1024                   # psum tile width (2 banks; one copy per out tile)

_cached = {}


def _build_program(G=G, xbufs=8, obufs=16, psbufs=4, psum_w=PSW):
    """Build (once per config) the per-core Bass program: out = W @ xs."""
    key = (G, xbufs, obufs, psbufs, psum_w)
    if key in _cached:
        return _cached[key]

    import concourse.bass as bass  # noqa: F401
    import concourse.tile as tile
    from concourse import bacc, mybir

    f32 = mybir.dt.float32
    nc = bacc.Bacc("TRN2", target_bir_lowering=False, debug=False)

    xs = nc.dram_tensor("xs", [2 * C_IN, HW], f32, kind="ExternalInput").ap()
    wa = nc.dram_tensor("wa", [2 * C_IN, C_OUT], f32, kind="ExternalInput").ap()
    wb = nc.dram_tensor("wb", [2 * C_IN, C_OUT], f32, kind="ExternalInput").ap()
    # out[b, m, o, hw] with o-chunk m of 128: host reshapes to [2, 256, HW]
    out = nc.dram_tensor(
        "out", [BPC, C_OUT // 128, 128, HW], f32, kind="ExternalOutput"
    ).ap()

    with tile.TileContext(nc) as tc:
        with (
            tc.tile_pool(name="w", bufs=1) as wpool,
            tc.tile_pool(name="xin", bufs=xbufs) as xpool,
            tc.tile_pool(name="ostage", bufs=obufs) as opool,
            tc.tile_pool(name="ps", bufs=psbufs, space="PSUM") as pspool,
        ):
            f32r = mybir.dt.float32r
            # fp32r operands must be produced as fp32r (verifier checks the
            # producer): cast-round during the SWDGE DMA load.
            wt = wpool.tile([128, 2, C_OUT], f32r)
            nc.gpsimd.dma_start(wt[:, 0], wa)
            nc.gpsimd.dma_start(wt[:, 1], wb)

            out_rings = [nc.scalar, nc.sync]
            di = 0
            for g in range(HW // G):
                xt = xpool.tile([128, G], f32r)
                nc.gpsimd.dma_start(xt, xs[:, g * G : (g + 1) * G])
                for b in range(BPC):
                    for m in range(C_OUT // 128):
                        ot = opool.tile([128, G], f32, tag="ot")
                        for s0 in range(max(G // psum_w, 1)):
                            pw = min(psum_w, G)
                            ps = pspool.tile([128, pw], f32)
                            for s1 in range(pw // 512):
                                col = s0 * pw + s1 * 512
                                # float32r = fp32 fast-matmul mode: full PE rate
                                # at moving dim >=256 (plain fp32 is 1/4 rate)
                                nc.tensor.matmul(
                                    ps[:, s1 * 512 : (s1 + 1) * 512],
                                    wt[:, b, m * 128 : (m + 1) * 128],
                                    xt[:, col : col + 512],
                                    start=True,
                                    stop=True,
                                )
                            nc.any.tensor_copy(ot[:, s0 * pw : (s0 + 1) * pw], ps)
                        out_rings[di % 2].dma_start(
                            out[b, m, :, g * G : (g + 1) * G], ot
                        )
                        di += 1

    nc.compile()
    _cached[key] = nc
    return nc


def _effective_weights(dictionary, lookup_coefficients, lookup_indices):
    """Fold conv dictionary + sparse combine into two padded lhsT weights."""
    idx = np.asarray(lookup_indices).reshape(C_OUT, -1).astype(np.int64)
    coeff = np.asarray(lookup_coefficients, np.float32).reshape(C_OUT, -1)
    w2 = np.zeros((C_OUT, D_SIZE), np.float32)
    np.add.at(w2, (np.arange(C_OUT)[:, None], idx), coeff)
    w_eff = w2 @ np.asarray(dictionary, np.float32).reshape(D_SIZE, C_IN)  # [O, C]
    wa = np.zeros((2 * C_IN, C_OUT), np.float32)
    wb = np.zeros((2 * C_IN, C_OUT), np.float32)
    wa[:C_IN] = w_eff.T
    wb[C_IN:] = w_eff.T
    return wa, wb, w_eff


def make_in_maps(x, dictionary, lookup_coefficients, lookup_indices):
    wa, wb, w_eff = _effective_weights(
        dictionary, lookup_coefficients, lookup_indices
    )
    xf = np.ascontiguousarray(np.asarray(x, np.float32).reshape(B, C_IN, HW))
    maps = [
        {
            "xs": np.ascontiguousarray(
                xf[i * BPC : (i + 1) * BPC].reshape(BPC * C_IN, HW)
            ),
            "wa": wa,
            "wb": wb,
        }
        for i in range(N_CORES)
    ]
    return maps, w_eff, xf


def _spot_check(out, w_eff, xf, rng):
    """Verify a random sample of outputs on the host (guards a rare
    first-execution flake seen on the PJRT path)."""
    n = 2048
    bs = rng.integers(0, B, n)
    os_ = rng.integers(0, C_OUT, n)
    ps = rng.integers(0, HW, n)
    ref = np.einsum("nc,nc->n", w_eff[os_], xf[bs, :, ps])
    got = out.reshape(B, C_OUT, HW)[bs, os_, ps]
    tol = 1e-4 * max(np.abs(ref).max(), 1.0)
    return np.all(np.isfinite(got)) and np.abs(got - ref).max() < tol


def kernel(x, dictionary, lookup_coefficients, lookup_indices):
    from concourse.bass_utils import run_bass_kernel_spmd

    nc = _build_program()
    in_maps, w_eff, xf = make_in_maps(
        x, dictionary, lookup_coefficients, lookup_indices
    )
    rng = np.random.default_rng(0)
    for _attempt in range(3):
        res = run_bass_kernel_spmd(nc, in_maps, core_ids=list(range(N_CORES)))
        out = np.concatenate(
            [res.results[i]["out"].reshape(BPC, C_OUT, H, W) for i in range(N_CORES)],
            axis=0,
        )
        if _spot_check(out, w_eff, xf, rng):
            break
    return out

